# revision 11
# baseline (speedup 1.0000x reference)
"""Biclique (GAT-style) attention layer on 8 Trainium2 NeuronCores.

Strategy (dst-sharded, edge-materialized, no collectives, no device gather):
  - Each core owns 6250 destination nodes (49 chunks of 128).
  - Host sorts edges by dst and buckets them per (core, 128-node chunk),
    padding each chunk to a uniform tile count t_ch.  Per chunk the host
    materializes
      * fT[chunk]  = feat[src[slots]].T as bf16, [128 feat, t_ch*128 slot]
      * OH[chunk]  = bf16 one-hot [128 slot, t_ch*128 (tile,node_local)]
    so each chunk needs exactly two large sequential DMAs.
  - Device, per 128-slot tile:  one matmul  y|score = ftile.T @ [Wm | Wm@A]
    ([slot, 132] in PSUM), one DVE op lin = 1 + 0.01*score, per PSUM-batch
    one scalar-engine  exp(100*lin - 100) = exp(score)  (only Exp is ever
    loaded -> no activation-table thrashing) blended as
    ex = max(exp(score), lin)  which equals  exp(leaky_relu(score))  to
    3.4e-4 relative, then g = y * ex via one broadcast multiply and two
    PSUM-accumulated matmuls (OH^T @ g, OH^T @ ex) for the segment sum and
    softmax denominator.  Epilogue: reciprocal + fused multiply-relu.
  - No segment-max subtraction (logits bounded ~[-0.03, 2.6]); bf16 data
    gives ~3e-3 relative error overall.
"""

import numpy as np

N = 50000
E = 800000
IN = 128
OUT = 128
H = 4
D = 32
P = 128
NCORES = 8
NODES_PER_CORE = N // NCORES               # 6250
N_CHUNKS = (NODES_PER_CORE + P - 1) // P   # 49
BT = 4                                     # proj-PSUM tiles per exp batch

_COMPILED = {}
LAST_RESULT = None


def _build_program(t_ch):
    import concourse.bass as bass
    import concourse.mybir as mybir
    import concourse.tile as tile
    from concourse import bacc
    from concourse.bass import AP

    f32 = mybir.dt.float32
    bf16 = mybir.dt.bfloat16
    W_SLOT = t_ch * P

    nc = bacc.Bacc("TRN2", target_bir_lowering=False, debug=False,
                   num_devices=NCORES)

    ft_t = nc.dram_tensor("ft", [N_CHUNKS, P, W_SLOT], bf16,
                          kind="ExternalInput").ap()
    oh_t = nc.dram_tensor("oh", [N_CHUNKS, P, W_SLOT], bf16,
                          kind="ExternalInput").ap()
    w_t = nc.dram_tensor("w", [IN, OUT], f32, kind="ExternalInput").ap()
    mask_t = nc.dram_tensor("mask", [IN, 1], f32, kind="ExternalInput").ap()
    attn_t = nc.dram_tensor("attn_rep", [P, OUT], f32, kind="ExternalInput").ap()
    out_t = nc.dram_tensor("out", [N_CHUNKS * P, OUT], f32,
                           kind="ExternalOutput").ap()

    with tile.TileContext(nc) as tc:
        with (
            tc.tile_pool(name="const", bufs=1) as cpool,
            tc.tile_pool(name="sbuf", bufs=4) as pool,
            tc.tile_pool(name="chk", bufs=2) as chpool,
            tc.tile_pool(name="psP", bufs=BT, space="PSUM") as psP,
            tc.tile_pool(name="psE", bufs=2, space="PSUM") as psE,
            tc.tile_pool(name="psD", bufs=2, space="PSUM") as psD,
        ):
            # Wcat = [W*mask | (W*mask) @ blockdiag(attn)] in bf16, [128, 132]
            w_sb = cpool.tile([IN, OUT], f32)
            nc.sync.dma_start(out=w_sb[:], in_=w_t[:])
            mask_sb = cpool.tile([IN, 1], f32)
            nc.sync.dma_start(out=mask_sb[:], in_=mask_t[:])
            nc.vector.tensor_scalar_mul(w_sb[:], w_sb[:], mask_sb[:, 0:1])
            attn_sb = cpool.tile([P, OUT], f32)
            nc.sync.dma_start(out=attn_sb[:], in_=attn_t[:])
            wa_tmp = cpool.tile([P, OUT], f32)
            nc.vector.tensor_tensor(out=wa_tmp[:], in0=w_sb[:], in1=attn_sb[:],
                                    op=mybir.AluOpType.mult)
            wa4 = cpool.tile([P, H], f32)
            nc.vector.tensor_reduce(
                out=wa4[:],
                in_=wa_tmp[:].rearrange("p (h d) -> p h d", d=D),
                axis=mybir.AxisListType.X, op=mybir.AluOpType.add)
            wcat = cpool.tile([IN, OUT + H], bf16)
            nc.vector.tensor_copy(out=wcat[:, OUT:OUT + H], in_=wa4[:])
            nc.vector.tensor_copy(out=wcat[:, 0:OUT], in_=w_sb[:])
            bias_m100 = cpool.tile([P, 1], f32)
            nc.vector.memset(bias_m100[:], -100.0)

            for j in range(N_CHUNKS):
                ft_ch = chpool.tile([P, W_SLOT], bf16, tag="ftch")
                nc.sync.dma_start(out=ft_ch[:], in_=ft_t[j])
                oh_ch = chpool.tile([P, W_SLOT], bf16, tag="ohch")
                nc.scalar.dma_start(out=oh_ch[:], in_=oh_t[j])
                lin_ch = chpool.tile([P, t_ch * H], f32, tag="linch")
                ex_ch = chpool.tile([P, t_ch * H], f32, tag="exch")
                ex_bf = chpool.tile([P, t_ch * H], bf16, tag="exbf")
                ps_num = psE.tile([P, OUT], f32)
                ps_den = psD.tile([P, H], f32)
                for b0 in range(0, t_ch, BT):
                    b1 = min(b0 + BT, t_ch)
                    ypss = {}
                    for t in range(b0, b1):
                        yps = psP.tile([P, OUT + H], f32)
                        nc.tensor.matmul(yps[:],
                                         lhsT=ft_ch[:, t * P:(t + 1) * P],
                                         rhs=wcat[:], start=True, stop=True)
                        # lin = 1 + 0.01*score (also evacuates score from PSUM)
                        nc.vector.tensor_scalar(
                            out=lin_ch[:, t * H:(t + 1) * H],
                            in0=yps[:, OUT:OUT + H],
                            scalar1=0.01, scalar2=1.0,
                            op0=mybir.AluOpType.mult, op1=mybir.AluOpType.add)
                        ypss[t] = yps
                    cols = slice(b0 * H, b1 * H)
                    # exp(100*lin-100) = exp(score); max(,lin) = leaky branch
                    nc.scalar.activation(
                        out=ex_ch[:, cols], in_=lin_ch[:, cols],
                        func=mybir.ActivationFunctionType.Exp,
                        bias=bias_m100[:, 0:1], scale=100.0)
                    nc.vector.tensor_tensor(
                        out=ex_ch[:, cols], in0=ex_ch[:, cols],
                        in1=lin_ch[:, cols], op=mybir.AluOpType.max)
                    nc.vector.tensor_copy(out=ex_bf[:, cols], in_=ex_ch[:, cols])
                    for t in range(b0, b1):
                        yps = ypss[t]
                        g_sb = pool.tile([P, OUT], bf16, tag="g")
                        ex_col = ex_ch[:, t * H:(t + 1) * H]
                        ex_bcast = AP(ex_col.tensor, ex_col.offset,
                                      [ex_col.ap[0], [ex_col.ap[1][0], H], [0, D]])
                        nc.vector.tensor_tensor(
                            out=g_sb[:].rearrange("p (h d) -> p h d", d=D),
                            in0=yps[:, 0:OUT].rearrange("p (h d) -> p h d", d=D),
                            in1=ex_bcast, op=mybir.AluOpType.mult)
                        nc.tensor.matmul(ps_num[:],
                                         lhsT=oh_ch[:, t * P:(t + 1) * P],
                                         rhs=g_sb[:],
                                         start=(t == 0), stop=(t == t_ch - 1))
                        nc.tensor.matmul(ps_den[:],
                                         lhsT=oh_ch[:, t * P:(t + 1) * P],
                                         rhs=ex_bf[:, t * H:(t + 1) * H],
                                         start=(t == 0), stop=(t == t_ch - 1))

                den = pool.tile([P, H], f32, tag="den")
                nc.vector.tensor_scalar_add(den[:], ps_den[:], 1e-30)
                rec = pool.tile([P, H], f32, tag="rec")
                nc.vector.reciprocal(out=rec[:], in_=den[:])
                ot = pool.tile([P, OUT], f32, tag="ot")
                for hh in range(H):
                    nc.vector.tensor_scalar(
                        out=ot[:, hh * D:(hh + 1) * D],
                        in0=ps_num[:, hh * D:(hh + 1) * D],
                        scalar1=rec[:, hh:hh + 1], scalar2=0.0,
                        op0=mybir.AluOpType.mult, op1=mybir.AluOpType.max)
                nc.sync.dma_start(out=out_t[j * P:(j + 1) * P, :], in_=ot[:])

    nc.compile()
    return nc


def _prep_edges(feat_bf, src, dst):
    """Sort by dst, bucket per (core, chunk), pad to uniform tile count.
    Returns (t_ch, fT[NCORES,N_CHUNKS,128,t_ch*128], OH[same])  (bf16)."""
    import ml_dtypes

    order = np.argsort(dst, kind="stable")
    src_s = src[order].astype(np.int64)
    dst_s = dst[order].astype(np.int64)

    core_edges = []
    t_ch = 1
    for c in range(NCORES):
        base = c * NODES_PER_CORE
        e0 = np.searchsorted(dst_s, base)
        e1 = np.searchsorted(dst_s, base + NODES_PER_CORE)
        cs, cd = src_s[e0:e1], dst_s[e0:e1]
        bnds = [np.searchsorted(cd, min(base + j * P, base + NODES_PER_CORE))
                for j in range(N_CHUNKS + 1)]
        core_edges.append((cs, cd, bnds))
        for j in range(N_CHUNKS):
            t_ch = max(t_ch, -(-(bnds[j + 1] - bnds[j]) // P))

    w_slot = t_ch * P
    slots_src = np.zeros((NCORES, N_CHUNKS, w_slot), np.int64)
    slots_dl = np.full((NCORES, N_CHUNKS, w_slot), P, np.int64)
    for c in range(NCORES):
        cs, cd, bnds = core_edges[c]
        base = c * NODES_PER_CORE
        for j in range(N_CHUNKS):
            cnt = bnds[j + 1] - bnds[j]
            slots_src[c, j, :cnt] = cs[bnds[j]:bnds[j + 1]]
            slots_dl[c, j, :cnt] = cd[bnds[j]:bnds[j + 1]] - (base + j * P)

    # fT[c, j] = feat_bf[slots].T  -> [128 feat, w_slot]
    fT = np.empty((NCORES, N_CHUNKS, P, w_slot), ml_dtypes.bfloat16)
    for c in range(NCORES):
        g = feat_bf[slots_src[c].reshape(-1)].reshape(N_CHUNKS, w_slot, IN)
        fT[c] = np.ascontiguousarray(g.transpose(0, 2, 1))

    # OH[c, j, s, t*128 + dl] = 1 for slot (t,s) with local dst dl
    oh_u16 = np.zeros((NCORES, N_CHUNKS, P, w_slot), np.uint16)
    one = np.float32(1.0).astype(ml_dtypes.bfloat16).view(np.uint16)
    cc, jj, ii = np.meshgrid(np.arange(NCORES), np.arange(N_CHUNKS),
                             np.arange(w_slot), indexing="ij")
    dl = slots_dl
    valid = dl < P
    tt = ii // P
    ss = ii % P
    oh_u16[cc[valid], jj[valid], ss[valid], tt[valid] * P + dl[valid]] = one
    OH = oh_u16.view(ml_dtypes.bfloat16)
    return t_ch, fT, OH


def kernel(feat, mask, W, attn_param, src, dst, _trace=False):
    global LAST_RESULT
    import ml_dtypes
    from concourse.bass_utils import run_bass_kernel_spmd

    feat = np.ascontiguousarray(np.asarray(feat, np.float32))
    mask = np.asarray(mask, np.float32)
    W = np.ascontiguousarray(np.asarray(W, np.float32))
    attn = np.asarray(attn_param, np.float32)
    src = np.asarray(src)
    dst = np.asarray(dst)

    feat_bf = feat.astype(ml_dtypes.bfloat16)
    t_ch, fT, OH = _prep_edges(feat_bf, src, dst)

    if t_ch not in _COMPILED:
        _COMPILED[t_ch] = _build_program(t_ch)
    nc = _COMPILED[t_ch]

    shared = {
        "w": W,
        "mask": mask.reshape(IN, 1).copy(),
        "attn_rep": np.tile(attn.reshape(1, OUT), (P, 1)).astype(np.float32),
    }
    in_maps = [
        {**shared, "ft": fT[c], "oh": OH[c]}
        for c in range(NCORES)
    ]
    res = None
    for attempt in range(3):
        try:
            res = run_bass_kernel_spmd(nc, in_maps, core_ids=list(range(NCORES)),
                                       trace=_trace)
            break
        except Exception as e:
            import traceback
            print(f"kernel: attempt {attempt} failed: {e!r}")
            traceback.print_exc()
            if attempt == 2:
                raise
    LAST_RESULT = res
    out = np.concatenate(
        [res.results[c]["out"][:NODES_PER_CORE] for c in range(NCORES)], axis=0)
    return out.astype(np.float32)
